# revision 1
# baseline (speedup 1.0000x reference)
"""Batched Kalman-gain kernel for Trainium2 (Bass/Tile), 8-core data parallel.

Per batch b (262144 of them):
    Sigma = F Sp F^T + Q            [8,8]
    S     = H Sigma H^T + R         [4,4]
    KG    = Sigma H^T S^-1          [8,4]

Factored to avoid materializing Sigma:
    A   = H F                       [4,8]
    C   = Sp A^T                    [8,4]
    P12 = F C + Q H^T  (= Sigma H^T) [8,4]
    S   = H P12 + R                 [4,4]
    X   = S^-1  (SPD, 2x2-block Schur complement)
    KG  = P12 X

Mapping: "planes" layout. 128 SBUF partitions = batch lanes; each lane holds
G consecutive batches' matrices along the free axis. Every per-batch product
is a wide elementwise tensor_tensor with broadcast access patterns
(DVE/GPSIMD). All contraction *sums* ride the TensorEngine for free: an
identity stationary operand (bitcast float32r -> 1 cycle/row) turns PSUM
accumulation across matmuls into elementwise tile summation. ScalarE (ACT)
evacuates PSUM->SBUF. The 4x4 SPD inverse is elementwise via the Schur
complement of the leading 2x2 block.

The per-chunk stages are software-pipelined with skew (engine queues are
FIFO: without skew, each engine stalls on the intra-chunk chain
A -> C -> P12 -> S -> X -> KG):
    iter t:  load(t) | A(t-1) | C,P12(t-2) | KG(t-4) | S,inv(t-3)
"""

import os

import numpy as np

P = 128          # SBUF partitions (batch lanes)
G = 32           # consecutive batches per lane per chunk
B = 262144       # full problem batch
NCORES = 8
B_CORE = B // NCORES           # 32768 per core
CHUNK = P * G                  # 4096 batches per chunk
NCHUNK = B_CORE // CHUNK       # 8 chunks

_NC_CACHE = {}


def _build_nc(b_core=B_CORE, g=G, repeat=1):
    import concourse.bacc as bacc
    import concourse.mybir as mybir
    import concourse.tile as tile
    from concourse.masks import make_identity

    fp32 = mybir.dt.float32
    fp32r = mybir.dt.float32r
    MULT = mybir.AluOpType.mult

    nchunk = b_core // (P * g)
    assert nchunk * P * g == b_core
    nc = bacc.Bacc("TRN2", target_bir_lowering=False, debug=False)

    F_d = nc.dram_tensor("F", [b_core, 8, 8], fp32, kind="ExternalInput").ap()
    H_d = nc.dram_tensor("H", [b_core, 4, 8], fp32, kind="ExternalInput").ap()
    Sp_d = nc.dram_tensor(
        "Sigma_previous", [b_core, 8, 8], fp32, kind="ExternalInput"
    ).ap()
    Q_d = nc.dram_tensor("Q", [b_core, 8, 8], fp32, kind="ExternalInput").ap()
    R_d = nc.dram_tensor("R", [b_core, 4, 4], fp32, kind="ExternalInput").ap()
    KG_d = nc.dram_tensor("KG", [b_core, 8, 4], fp32, kind="ExternalOutput").ap()

    # chunk views: batch = c*(P*g) + p*g + g_idx  (lane-contiguous DMA)
    Fv = F_d.rearrange("(c p g) i j -> c p g i j", p=P, g=g)
    Hv = H_d.rearrange("(c p g) m j -> c p g m j", p=P, g=g)
    Spv = Sp_d.rearrange("(c p g) i j -> c p g i j", p=P, g=g)
    Qv = Q_d.rearrange("(c p g) i j -> c p g i j", p=P, g=g)
    Rv = R_d.rearrange("(c p g) m n -> c p g m n", p=P, g=g)
    KGv = KG_d.rearrange("(c p g) i m -> c p g i m", p=P, g=g)

    BANK = 512  # fp32 elems per PSUM bank per partition
    all_dve = os.environ.get("ALL_DVE", "0") == "1"

    with tile.TileContext(nc) as tc:
        with (
            tc.tile_pool(name="consts", bufs=1) as consts,
            tc.tile_pool(name="ins3", bufs=3) as insp,
            tc.tile_pool(name="ins4", bufs=4) as insp2,
            tc.tile_pool(name="mid2", bufs=2) as midp,
            tc.tile_pool(name="mid3", bufs=3) as midp3,
            tc.tile_pool(name="prod", bufs=4) as prodp,
            tc.tile_pool(name="inv", bufs=2) as invp,
            tc.tile_pool(name="psum", bufs=8, space="PSUM") as psump,
        ):
            ident = consts.tile([P, P], fp32, tag="ident")
            make_identity(nc, ident[:])
            identr_t = consts.tile([P, P], fp32r, tag="identr")
            nc.vector.tensor_copy(identr_t[:], ident[:])
            identr = identr_t[:]

            def flat(t):
                return t[:].rearrange("p g a b -> p (g a b)")

            def contract(terms, out_tag, width, pool, extra_rhs=None):
                """terms: list of (engine, in0_ap, in1_ap, prod_shape[2:]).
                Returns SBUF tile [P, g, a, b] = sum of products (+extra_rhs).
                Products are elementwise TT ops; the sum runs on the PE via
                float32r identity matmuls accumulating in PSUM."""
                npc = (width + BANK - 1) // BANK
                rhs_list = []
                for eng, a_ap, b_ap, (d0, d1) in terms:
                    etag = "prodv" if eng is nc.vector else "prodg"
                    pt = prodp.tile([P, g, d0, d1], fp32r, tag=etag, name=etag)
                    eng.tensor_tensor(pt[:], a_ap, b_ap, op=MULT)
                    rhs_list.append(flat(pt))
                if extra_rhs is not None:
                    rhs_list.append(extra_rhs)
                d0, d1 = terms[0][3]
                out = pool.tile([P, g, d0, d1], fp32, tag=out_tag, name=out_tag)
                outf = flat(out)
                ps_tiles = [
                    psump.tile([P, BANK], fp32, tag="ps", name=f"ps_{out_tag}_{pc}")
                    for pc in range(npc)
                ]
                nterm = len(rhs_list)
                for pc in range(npc):
                    lo, hi = pc * BANK, min((pc + 1) * BANK, width)
                    for t, rhs in enumerate(rhs_list):
                        nc.tensor.matmul(
                            ps_tiles[pc][:, : hi - lo],
                            identr,
                            rhs[:, lo:hi],
                            start=(t == 0),
                            stop=(t == nterm - 1),
                        )
                    nc.scalar.copy(outf[:, lo:hi], ps_tiles[pc][:, : hi - lo])
                return out

            def bc(ap, axis, shape):
                return ap.unsqueeze(axis).broadcast_to(shape)

            st = [dict() for _ in range(nchunk)]
            V = nc.vector
            GP = nc.vector if all_dve else nc.gpsimd
            sh48 = [P, g, 4, 8]
            sh84 = [P, g, 8, 4]
            sh44 = [P, g, 4, 4]
            sh22 = [P, g, 2, 2]

            def emit_load(c):
                s = st[c]
                s["F"] = insp.tile([P, g, 8, 8], fp32, tag="F", name="Ft")
                s["Sp"] = insp.tile([P, g, 8, 8], fp32, tag="Sp", name="Spt")
                s["Q"] = insp.tile([P, g, 8, 8], fp32, tag="Q", name="Qt")
                s["H"] = insp2.tile([P, g, 4, 8], fp32, tag="H", name="Ht")
                s["R"] = insp2.tile([P, g, 4, 4], fp32, tag="R", name="Rt")
                nc.sync.dma_start(out=s["F"][:], in_=Fv[c])
                nc.sync.dma_start(out=s["H"][:], in_=Hv[c])
                nc.sync.dma_start(out=s["Sp"][:], in_=Spv[c])
                nc.sync.dma_start(out=s["Q"][:], in_=Qv[c])
                nc.sync.dma_start(out=s["R"][:], in_=Rv[c])

            def emit_A(c):
                s = st[c]
                Ft, Ht = s["F"], s["H"]
                s["A"] = contract(
                    [
                        (V, bc(Ht[:, :, :, j], 3, sh48), bc(Ft[:, :, j, :], 2, sh48), (4, 8))
                        for j in range(8)
                    ],
                    "A",
                    g * 32,
                    midp,
                )

            def emit_CP(c):
                s = st[c]
                Ft, Spt, Qt, Ht, A = s["F"], s["Sp"], s["Q"], s["H"], s["A"]
                C = contract(
                    [
                        (V, bc(Spt[:, :, :, k], 3, sh84), bc(A[:, :, :, k], 2, sh84), (8, 4))
                        for k in range(8)
                    ],
                    "C",
                    g * 32,
                    midp,
                )
                # Q*H^T terms first: they don't depend on C, so they fill the
                # engine while C is still accumulating/evacuating.
                s["P12"] = contract(
                    [
                        (GP, bc(Qt[:, :, :, j], 3, sh84), bc(Ht[:, :, :, j], 2, sh84), (8, 4))
                        for j in range(8)
                    ]
                    + [
                        (V, bc(Ft[:, :, :, j], 3, sh84), bc(C[:, :, j, :], 2, sh84), (8, 4))
                        for j in range(8)
                    ],
                    "P12",
                    g * 32,
                    midp3,
                )

            def emit_SX(c):
                s = st[c]
                Ht, Rt, P12 = s["H"], s["R"], s["P12"]
                Rr = prodp.tile([P, g, 4, 4], fp32r, tag="prodg", name="Rr")
                GP.tensor_copy(Rr[:], Rt[:])
                S = contract(
                    [
                        (GP, bc(Ht[:, :, :, i], 3, sh44), bc(P12[:, :, i, :], 2, sh44), (4, 4))
                        for i in range(8)
                    ],
                    "S",
                    g * 16,
                    midp,
                    extra_rhs=flat(Rr),
                )
                # ---- X = S^-1 via Schur complement of leading 2x2 block ----
                X = midp.tile([P, g, 4, 4], fp32, tag="X", name="X")
                Pi = invp.tile([P, g, 2, 2], fp32, tag="Pi", name="Pi")
                W = invp.tile([P, g, 2, 2], fp32, tag="W", name="W")
                Sc = invp.tile([P, g, 2, 2], fp32, tag="Sc", name="Sc")
                t3 = invp.tile([P, g, 2, 2], fp32, tag="t3", name="t3")
                t4 = invp.tile([P, g, 2, 2], fp32, tag="t4", name="t4")
                t5 = invp.tile([P, g, 2, 2], fp32, tag="t5", name="t5")
                t7 = invp.tile([P, g, 2, 2], fp32, tag="t7", name="t7")
                d0 = invp.tile([P, g], fp32, tag="d0", name="d0")
                u0 = invp.tile([P, g], fp32, tag="u0", name="u0")
                u1 = invp.tile([P, g], fp32, tag="u1", name="u1")
                r0 = invp.tile([P, g], fp32, tag="r0", name="r0")
                nr0 = invp.tile([P, g], fp32, tag="nr0", name="nr0")
                d1 = invp.tile([P, g], fp32, tag="d1", name="d1")
                r1 = invp.tile([P, g], fp32, tag="r1", name="r1")
                nr1 = invp.tile([P, g], fp32, tag="nr1", name="nr1")

                sa = S[:, :, 0, 0]
                sb = S[:, :, 0, 1]
                sb2 = S[:, :, 1, 0]
                sc_ = S[:, :, 1, 1]
                V.tensor_mul(u0[:], sa, sc_)
                V.tensor_mul(u1[:], sb, sb2)
                V.tensor_sub(d0[:], u0[:], u1[:])
                V.reciprocal(r0[:], d0[:])
                V.tensor_scalar_mul(nr0[:], r0[:], -1.0)
                V.tensor_mul(Pi[:, :, 0, 0], sc_, r0[:])
                V.tensor_mul(Pi[:, :, 1, 1], sa, r0[:])
                V.tensor_mul(Pi[:, :, 0, 1], sb, nr0[:])
                V.tensor_mul(Pi[:, :, 1, 0], sb2, nr0[:])
                # W = B Aq^-1
                V.tensor_mul(
                    W[:], bc(S[:, :, 2:4, 0], 3, sh22), bc(Pi[:, :, 0, :], 2, sh22)
                )
                V.tensor_mul(
                    t4[:], bc(S[:, :, 2:4, 1], 3, sh22), bc(Pi[:, :, 1, :], 2, sh22)
                )
                V.tensor_add(W[:], W[:], t4[:])
                # Sc = D - W B^T
                V.tensor_mul(
                    t3[:], bc(W[:, :, :, 0], 3, sh22), bc(S[:, :, 2:4, 0], 2, sh22)
                )
                V.tensor_mul(
                    t4[:], bc(W[:, :, :, 1], 3, sh22), bc(S[:, :, 2:4, 1], 2, sh22)
                )
                V.tensor_add(t3[:], t3[:], t4[:])
                V.tensor_sub(Sc[:], S[:, :, 2:4, 2:4], t3[:])
                # Sc^-1 -> X[2:4,2:4]
                V.tensor_mul(u0[:], Sc[:, :, 0, 0], Sc[:, :, 1, 1])
                V.tensor_mul(u1[:], Sc[:, :, 0, 1], Sc[:, :, 1, 0])
                V.tensor_sub(d1[:], u0[:], u1[:])
                V.reciprocal(r1[:], d1[:])
                V.tensor_scalar_mul(nr1[:], r1[:], -1.0)
                V.tensor_mul(X[:, :, 2, 2], Sc[:, :, 1, 1], r1[:])
                V.tensor_mul(X[:, :, 3, 3], Sc[:, :, 0, 0], r1[:])
                V.tensor_mul(X[:, :, 2, 3], Sc[:, :, 0, 1], nr1[:])
                V.tensor_mul(X[:, :, 3, 2], Sc[:, :, 1, 0], nr1[:])
                # X21 = -Sc^-1 W -> X[2:4,0:2]
                V.tensor_mul(
                    t5[:], bc(X[:, :, 2:4, 2], 3, sh22), bc(W[:, :, 0, :], 2, sh22)
                )
                V.tensor_mul(
                    t4[:], bc(X[:, :, 2:4, 3], 3, sh22), bc(W[:, :, 1, :], 2, sh22)
                )
                V.tensor_add(t5[:], t5[:], t4[:])
                V.tensor_scalar_mul(X[:, :, 2:4, 0:2], t5[:], -1.0)
                # X12 = X21^T
                V.tensor_copy(
                    X[:, :, 0:2, 2:4], X[:, :, 2:4, 0:2].transpose([0, 1, 3, 2])
                )
                # X11 = Aq^-1 - W^T X21
                V.tensor_mul(
                    t7[:], bc(W[:, :, 0, :], 3, sh22), bc(X[:, :, 2, 0:2], 2, sh22)
                )
                V.tensor_mul(
                    t4[:], bc(W[:, :, 1, :], 3, sh22), bc(X[:, :, 3, 0:2], 2, sh22)
                )
                V.tensor_add(t7[:], t7[:], t4[:])
                V.tensor_sub(X[:, :, 0:2, 0:2], Pi[:], t7[:])
                s["X"] = X

            def emit_KG(c):
                s = st[c]
                P12, X = s["P12"], s["X"]
                # term order 2,3,0,1: X rows 2,3 (Sc^-1, X21) are written
                # before rows 0,1, so the PE series can start earlier.
                KG = contract(
                    [
                        (V, bc(P12[:, :, :, n], 3, sh84), bc(X[:, :, n, :], 2, sh84), (8, 4))
                        for n in (2, 3, 0, 1)
                    ],
                    "KG",
                    g * 32,
                    midp,
                )
                nc.sync.dma_start(out=KGv[c], in_=KG[:])

            def emit_all():
                for t in range(nchunk + 4):
                    if t < nchunk:
                        emit_load(t)
                    if 0 <= t - 1 < nchunk:
                        emit_A(t - 1)
                    if 0 <= t - 2 < nchunk:
                        emit_CP(t - 2)
                    if 0 <= t - 4 < nchunk:
                        emit_KG(t - 4)
                    if 0 <= t - 3 < nchunk:
                        emit_SX(t - 3)

            if repeat > 1:
                with tc.For_i(0, repeat, 1):
                    emit_all()
            else:
                emit_all()

    nc.compile()
    return nc


def _get_nc():
    if "nc" not in _NC_CACHE:
        _NC_CACHE["nc"] = _build_nc()
    return _NC_CACHE["nc"]


def kernel(F, H, Sigma_previous, Q, R):
    from concourse.bass_utils import run_bass_kernel_spmd

    nc = _get_nc()
    in_maps = []
    for ci in range(NCORES):
        sl = slice(ci * B_CORE, (ci + 1) * B_CORE)
        in_maps.append(
            {
                "F": np.ascontiguousarray(F[sl], dtype=np.float32),
                "H": np.ascontiguousarray(H[sl], dtype=np.float32),
                "Sigma_previous": np.ascontiguousarray(
                    Sigma_previous[sl], dtype=np.float32
                ),
                "Q": np.ascontiguousarray(Q[sl], dtype=np.float32),
                "R": np.ascontiguousarray(R[sl], dtype=np.float32),
            }
        )
    res = run_bass_kernel_spmd(nc, in_maps, core_ids=list(range(NCORES)))
    return np.concatenate([r["KG"] for r in res.results], axis=0)


# KG term n needs X row n; the (2, 3, 0, 1) order in emit_KG matches the
# order X's rows become available. The n-th PE accumulation term uses the
# same reordering, which is sum-order-irrelevant.



# revision 6
# speedup vs baseline: 2.2564x; 2.2564x over previous
"""Batched Kalman-gain kernel for Trainium2 (Bass/Tile), 8-core data parallel.

Per batch b (262144 of them):
    Sigma = F Sp F^T + Q            [8,8]
    S     = H Sigma H^T + R         [4,4]
    KG    = Sigma H^T S^-1          [8,4]

Factored (A = H F, U = A Sp, HQ = H Q):
    P12 = F U^T + (HQ)^T  (= Sigma H^T)  [8,4]
    S   = H P12 + R                      [4,4]
    X   = S^-1  (SPD, 2x2-block Schur complement, fp32)
    KG  = P12 X                          [8,4]

Layout: "batch-innermost planes". The HOST pre-transposes every input to
[chunk, P, d0, d1, g] (P=128 SBUF partitions, g consecutive batches
innermost) and casts to fp16. On device every per-batch product is a wide
elementwise DVE tensor_tensor in fp16 — the innermost g axis is stride-1 and
>=2 elements on all operands, which enables the DVE 16-bit 2x mode
(measured 0.60 ns/free-elem vs 1.19 for fp32). Contraction sums for
A/U/HQ/P12 ride the TensorEngine (fp16 identity stationary, fp32 PSUM
accumulate; ScalarE evacuates PSUM -> fp16). S and KG sums are pairwise
fp16 adds on DVE. The 4x4 SPD inverse runs in fp32 on DVE, batched over 4
chunks so per-op fixed overhead (~200ns) amortizes.

Numerics (validated against a float64 reference in numpy): fp16 inputs +
fp16 products + fp32 PSUM sums + fp16 intermediates + fp32 S/inverse +
fp16 X/KG gives rel err ~3e-3 (tolerance 2e-2).
"""

import numpy as np

P = 128
G = 32
B = 262144
NCORES = 8
B_CORE = B // NCORES           # 32768
CHUNK = P * G                  # 4096
NCHUNK = B_CORE // CHUNK       # 8
IBATCH = 4                     # chunks per inverse batch

_NC_CACHE = {}


def _build_nc():
    import concourse.bacc as bacc
    import concourse.mybir as mybir
    import concourse.tile as tile
    from concourse.masks import make_identity

    fp32 = mybir.dt.float32
    fp16 = mybir.dt.float16
    MULT = mybir.AluOpType.mult
    ADD = mybir.AluOpType.add
    SUB = mybir.AluOpType.subtract
    COPY = mybir.ActivationFunctionType.Copy

    nc = bacc.Bacc("TRN2", target_bir_lowering=False, debug=False)

    F_d = nc.dram_tensor("F", [NCHUNK, P, 8, 8, G], fp16, kind="ExternalInput").ap()
    Sp_d = nc.dram_tensor(
        "Sigma_previous", [NCHUNK, P, 8, 8, G], fp16, kind="ExternalInput"
    ).ap()
    Q_d = nc.dram_tensor("Q", [NCHUNK, P, 8, 8, G], fp16, kind="ExternalInput").ap()
    H_d = nc.dram_tensor("H", [NCHUNK, P, 4, 8, G], fp16, kind="ExternalInput").ap()
    R_d = nc.dram_tensor("R", [NCHUNK, P, 4, 4, G], fp16, kind="ExternalInput").ap()
    KG_d = nc.dram_tensor("KG", [NCHUNK, P, 8, 4, G], fp16, kind="ExternalOutput").ap()

    NB = IBATCH

    with tile.TileContext(nc) as tc:
        with (
            tc.tile_pool(name="consts", bufs=1) as consts,
            tc.tile_pool(name="inF", bufs=4) as poolF,
            tc.tile_pool(name="inH", bufs=5) as poolH,
            tc.tile_pool(name="inSQ", bufs=3) as poolSQ,
            tc.tile_pool(name="inR", bufs=5) as poolR,
            tc.tile_pool(name="pprod", bufs=3) as pprod,
            tc.tile_pool(name="tprod", bufs=2) as tprod,
            tc.tile_pool(name="interm", bufs=2) as interm,
            tc.tile_pool(name="p12p", bufs=6) as p12p,
            tc.tile_pool(name="sx", bufs=2) as sxp,
            tc.tile_pool(name="inv", bufs=1) as invp,
            tc.tile_pool(name="out", bufs=2) as outp,
            tc.tile_pool(name="psum", bufs=8, space="PSUM") as psump,
        ):
            identf = consts.tile([P, P], fp32, tag="identf")
            make_identity(nc, identf[:])
            identh_t = consts.tile([P, P], fp16, tag="identh")
            nc.vector.tensor_copy(identh_t[:], identf[:])
            identh = identh_t[:]

            V = nc.vector
            ACT = nc.scalar

            def bc(ap, axis, shape):
                return ap.unsqueeze(axis).broadcast_to(shape)

            st = [dict() for _ in range(NCHUNK)]
            inv_st = [dict() for _ in range(NCHUNK // NB)]

            sh48 = [P, 4, 8, G]
            sh84 = [P, 8, 4, G]
            sh44 = [P, 4, 4, G]

            def emit_load(c):
                s = st[c]
                s["F"] = poolF.tile([P, 8, 8, G], fp16, tag="F", name="Ft")
                s["Sp"] = poolSQ.tile([P, 8, 8, G], fp16, tag="Sp", name="Spt")
                s["Q"] = poolSQ.tile([P, 8, 8, G], fp16, tag="Q", name="Qt")
                s["H"] = poolH.tile([P, 4, 8, G], fp16, tag="H", name="Ht")
                s["R"] = poolR.tile([P, 4, 4, G], fp16, tag="R", name="Rt")
                nc.sync.dma_start(out=s["F"][:], in_=F_d[c])
                nc.sync.dma_start(out=s["H"][:], in_=H_d[c])
                nc.sync.dma_start(out=s["Sp"][:], in_=Sp_d[c])
                nc.sync.dma_start(out=s["Q"][:], in_=Q_d[c])
                nc.sync.dma_start(out=s["R"][:], in_=R_d[c])

            def banks(ap4):
                # [P, d0, d1, G] -> two 512-elem bank APs (d0*d1*G == 1024)
                d0 = ap4.shape[1]
                return (ap4[:, : d0 // 2], ap4[:, d0 // 2 :])

            def pe_contract(slots, out_tag, extra_moving=None):
                """slots: list of (bank0_ap, bank1_ap) fp16 product AP pairs,
                each bank 512 free elems. extra_moving: same, appended to the
                PSUM accumulation (e.g. a transposed HQ pass).
                Returns fp16 SBUF tile [P, 1024] flat (caller reshapes)."""
                movings = list(slots) + (list(extra_moving) if extra_moving else [])
                nterm = len(movings)
                ps = [
                    psump.tile([P, 512], fp32, tag="ps", name=f"ps_{out_tag}_{b}")
                    for b in range(2)
                ]
                for t, mv in enumerate(movings):
                    for b in range(2):
                        nc.tensor.matmul(
                            ps[b][:, :],
                            identh,
                            mv[b],
                            start=(t == 0),
                            stop=(t == nterm - 1),
                        )
                pool = p12p if out_tag == "P12" else interm
                out = pool.tile([P, 1024], fp16, tag=out_tag, name=out_tag)
                for b in range(2):
                    ACT.activation(
                        out[:][:, b * 512 : (b + 1) * 512], ps[b][:, :], COPY
                    )
                return out

            def emit_A(c):
                s = st[c]
                Ft, Ht = s["F"], s["H"]
                slots = []
                for j in range(8):
                    pt = pprod.tile(sh48, fp16, tag="pA", name="pA")
                    V.tensor_tensor(
                        pt[:],
                        bc(Ht[:, :, j, :], 2, sh48),
                        bc(Ft[:, j, :, :], 1, sh48),
                        op=MULT,
                    )
                    slots.append(banks(pt[:]))
                s["A"] = pe_contract(slots, "A")  # A[m,k] flat [P, (m k g)]

            def emit_UHQ(c):
                s = st[c]
                Spt, Qt, Ht = s["Sp"], s["Q"], s["H"]
                Av = s["A"][:].rearrange("p (m k g) -> p m k g", m=4, k=8)
                slots = []
                for k in range(8):
                    pt = pprod.tile(sh48, fp16, tag="pU", name="pU")
                    V.tensor_tensor(
                        pt[:],
                        bc(Av[:, :, k, :], 2, sh48),
                        bc(Spt[:, k, :, :], 1, sh48),
                        op=MULT,
                    )
                    slots.append(banks(pt[:]))
                s["U"] = pe_contract(slots, "U")  # U[m,j]
                slots = []
                for j in range(8):
                    pt = pprod.tile(sh48, fp16, tag="pHQ", name="pHQ")
                    V.tensor_tensor(
                        pt[:],
                        bc(Ht[:, :, j, :], 2, sh48),
                        bc(Qt[:, j, :, :], 1, sh48),
                        op=MULT,
                    )
                    slots.append(banks(pt[:]))
                s["HQ"] = pe_contract(slots, "HQ")  # HQ[m,k]

            def emit_P12(c):
                s = st[c]
                Ft = s["F"]
                Uv = s["U"][:].rearrange("p (m j g) -> p m j g", m=4, j=8)
                slots = []
                for j in range(8):
                    pt = pprod.tile(sh84, fp16, tag="pP", name="pP")
                    V.tensor_tensor(
                        pt[:],
                        bc(Ft[:, :, j, :], 2, sh84),
                        bc(Uv[:, :, j, :], 1, sh84),
                        op=MULT,
                    )
                    slots.append(banks(pt[:]))
                # (HQ)^T pass: HQ[m,k] read as [k, m, g]
                hqt = (
                    s["HQ"][:]
                    .rearrange("p (m k g) -> p m k g", m=4, k=8)
                    .transpose([0, 2, 1, 3])
                )
                s["P12"] = pe_contract(
                    slots, "P12", extra_moving=[banks(hqt)]
                )

            def emit_S(c):
                s = st[c]
                Ht, Rt = s["H"], s["R"]
                P12v = s["P12"][:].rearrange("p (i m g) -> p i m g", i=8, m=4)
                sp = tprod.tile([P, 8, 4, 4, G], fp16, tag="spr", name="spr")
                for i in range(8):
                    V.tensor_tensor(
                        sp[:, i],
                        bc(Ht[:, :, i, :], 2, sh44),
                        bc(P12v[:, i, :, :], 1, sh44),
                        op=MULT,
                    )
                spf = sp[:].rearrange("p s m n g -> p s (m n g)")
                t1 = tprod.tile([P, 4, 512], fp16, tag="st1", name="st1")
                V.tensor_tensor(t1[:], spf[:, 0:4], spf[:, 4:8], op=ADD)
                t2 = tprod.tile([P, 2, 512], fp16, tag="st2", name="st2")
                V.tensor_tensor(t2[:], t1[:][:, 0:2], t1[:][:, 2:4], op=ADD)
                t3 = tprod.tile([P, 512], fp16, tag="st3", name="st3")
                V.tensor_tensor(t3[:], t2[:][:, 0], t2[:][:, 1], op=ADD)
                # S4 slot (fp32) = t3 + R
                k, ci = c // NB, c % NB
                if ci == 0:
                    inv_st[k]["S4"] = sxp.tile(
                        [P, 4, 4, NB, G], fp32, tag="S4", name="S4"
                    )
                S4 = inv_st[k]["S4"]
                V.tensor_tensor(
                    S4[:, :, :, ci, :],
                    t3[:].rearrange("p (m n g) -> p m n g", m=4, n=4),
                    Rt[:],
                    op=ADD,
                )

            def emit_INV(k):
                """X4 = S4^-1 via Schur complement of leading 2x2 block.
                S treated as symmetric (s10 := s01). All internals fp32,
                X4 written fp16. Ops are [P, NB, G]- or [P, 2, 2, NB, G]-sized.
                Scratch tags are reused across steps (bufs=1 pool) to bound
                SBUF: ta/tb/td/tr/tnr are [P,NB,G]; Pi/W/Sc/Si/x2a/x2b/x2c
                are [P,2,2,NB,G]."""
                s = inv_st[k]
                S4 = s["S4"]
                X4 = sxp.tile([P, 4, 4, NB, G], fp16, tag="X4", name="X4")
                s["X4"] = X4
                sh1 = [P, NB, G]
                sh2 = [P, 2, 2, NB, G]

                def t1(tag):
                    return invp.tile(sh1, fp32, tag=tag, name=tag)

                def t2(tag):
                    return invp.tile(sh2, fp32, tag=tag, name=tag)

                sa, sb, sc_ = S4[:, 0, 0], S4[:, 0, 1], S4[:, 1, 1]
                ta, tb, td, tr, tnr = t1("ta"), t1("tb"), t1("td"), t1("tr"), t1("tnr")
                V.tensor_tensor(ta[:], sa, sc_, op=MULT)
                V.tensor_tensor(tb[:], sb, sb, op=MULT)
                V.tensor_tensor(td[:], ta[:], tb[:], op=SUB)
                V.reciprocal(tr[:], td[:])
                V.tensor_scalar_mul(tnr[:], tr[:], -1.0)
                Pi = t2("Pi")
                V.tensor_tensor(Pi[:, 0, 0], sc_, tr[:], op=MULT)
                V.tensor_tensor(Pi[:, 1, 1], sa, tr[:], op=MULT)
                V.tensor_tensor(Pi[:, 0, 1], sb, tnr[:], op=MULT)
                V.tensor_copy(Pi[:, 1, 0], Pi[:, 0, 1])
                # B = S[2:4, 0:2];  W = B Pi
                Bq = S4[:, 2:4, 0:2]  # [P, 2, 2, NB, G]
                W, x2a = t2("W"), t2("x2a")
                V.tensor_tensor(
                    x2a[:], bc(Bq[:, :, 0], 2, sh2), bc(Pi[:, 0, :], 1, sh2), op=MULT
                )
                V.tensor_tensor(
                    W[:], bc(Bq[:, :, 1], 2, sh2), bc(Pi[:, 1, :], 1, sh2), op=MULT
                )
                V.tensor_tensor(W[:], W[:], x2a[:], op=ADD)
                # Sc = S[2:4,2:4] - W B^T
                x2b, Sc = t2("x2b"), t2("Sc")
                V.tensor_tensor(
                    x2a[:], bc(W[:, :, 0], 2, sh2), bc(Bq[:, :, 0], 1, sh2), op=MULT
                )
                V.tensor_tensor(
                    x2b[:], bc(W[:, :, 1], 2, sh2), bc(Bq[:, :, 1], 1, sh2), op=MULT
                )
                V.tensor_tensor(x2a[:], x2a[:], x2b[:], op=ADD)
                V.tensor_tensor(Sc[:], S4[:, 2:4, 2:4], x2a[:], op=SUB)
                # Si = Sc^-1 (fp32), X[2:4,2:4] = Si (fp16)
                V.tensor_tensor(ta[:], Sc[:, 0, 0], Sc[:, 1, 1], op=MULT)
                V.tensor_tensor(tb[:], Sc[:, 0, 1], Sc[:, 1, 0], op=MULT)
                V.tensor_tensor(td[:], ta[:], tb[:], op=SUB)
                V.reciprocal(tr[:], td[:])
                V.tensor_scalar_mul(tnr[:], tr[:], -1.0)
                Si = t2("Si")
                V.tensor_tensor(Si[:, 0, 0], Sc[:, 1, 1], tr[:], op=MULT)
                V.tensor_tensor(Si[:, 1, 1], Sc[:, 0, 0], tr[:], op=MULT)
                V.tensor_tensor(Si[:, 0, 1], Sc[:, 0, 1], tnr[:], op=MULT)
                V.tensor_copy(Si[:, 1, 0], Si[:, 0, 1])
                V.tensor_copy(X4[:, 2:4, 2:4], Si[:])
                # X21 = -(Si W)  -> X[2:4, 0:2];  X12 = X21^T
                x2c = t2("x2c")
                V.tensor_tensor(
                    x2a[:], bc(Si[:, :, 0], 2, sh2), bc(W[:, 0, :], 1, sh2), op=MULT
                )
                V.tensor_tensor(
                    x2b[:], bc(Si[:, :, 1], 2, sh2), bc(W[:, 1, :], 1, sh2), op=MULT
                )
                V.tensor_tensor(x2a[:], x2a[:], x2b[:], op=ADD)
                V.tensor_scalar_mul(X4[:, 2:4, 0:2], x2a[:], -1.0)
                V.tensor_copy(
                    X4[:, 0:2, 2:4], X4[:, 2:4, 0:2].transpose([0, 2, 1, 3, 4])
                )
                # X11 = Pi + W^T (Si W);  (Si W) is x2a
                V.tensor_tensor(
                    x2b[:], bc(W[:, 0, :], 2, sh2), bc(x2a[:, 0, :], 1, sh2), op=MULT
                )
                V.tensor_tensor(
                    x2c[:], bc(W[:, 1, :], 2, sh2), bc(x2a[:, 1, :], 1, sh2), op=MULT
                )
                V.tensor_tensor(x2b[:], x2b[:], x2c[:], op=ADD)
                V.tensor_tensor(X4[:, 0:2, 0:2], Pi[:], x2b[:], op=ADD)

            def emit_KG(c):
                s = st[c]
                k, ci = c // NB, c % NB
                X4 = inv_st[k]["X4"]
                P12v = s["P12"][:].rearrange("p (i m g) -> p i m g", i=8, m=4)
                kp = tprod.tile([P, 4, 8, 4, G], fp16, tag="kpr", name="kpr")
                for m in range(4):
                    V.tensor_tensor(
                        kp[:, m],
                        bc(P12v[:, :, m, :], 2, sh84),
                        bc(X4[:, m, :, ci, :], 1, sh84),
                        op=MULT,
                    )
                kpf = kp[:].rearrange("p s i n g -> p s (i n g)")
                k1 = tprod.tile([P, 2, 1024], fp16, tag="kt1", name="kt1")
                V.tensor_tensor(k1[:], kpf[:, 0:2], kpf[:, 2:4], op=ADD)
                KGh = outp.tile([P, 8, 4, G], fp16, tag="KGh", name="KGh")
                V.tensor_tensor(
                    KGh[:].rearrange("p i n g -> p (i n g)"),
                    k1[:][:, 0],
                    k1[:][:, 1],
                    op=ADD,
                )
                nc.sync.dma_start(out=KG_d[c], in_=KGh[:])

            for t in range(NCHUNK + 9):
                if t < NCHUNK:
                    emit_load(t)
                if 0 <= t - 1 < NCHUNK:
                    emit_A(t - 1)
                if 0 <= t - 2 < NCHUNK:
                    emit_UHQ(t - 2)
                if 0 <= t - 3 < NCHUNK:
                    emit_P12(t - 3)
                if 0 <= t - 4 < NCHUNK:
                    emit_S(t - 4)
                if 0 <= t - 4 < NCHUNK and (t - 4) % NB == NB - 1:
                    emit_INV((t - 4) // NB)
                if 0 <= t - 8 < NCHUNK:
                    emit_KG(t - 8)

    nc.compile()
    return nc


def _get_nc():
    if "nc" not in _NC_CACHE:
        _NC_CACHE["nc"] = _build_nc()
    return _NC_CACHE["nc"]


def prepare_in_map(F, H, Sigma_previous, Q, R, core):
    """Host-side shard + layout transform + fp16 cast for one core."""
    sl = slice(core * B_CORE, (core + 1) * B_CORE)

    def bi(x, d0, d1):
        # [B_CORE, d0, d1] -> [NCHUNK, P, d0, d1, G]
        v = x[sl].reshape(NCHUNK, P, G, d0, d1).transpose(0, 1, 3, 4, 2)
        return np.ascontiguousarray(v, dtype=np.float16)

    return {
        "F": bi(F, 8, 8),
        "Sigma_previous": bi(Sigma_previous, 8, 8),
        "Q": bi(Q, 8, 8),
        "H": bi(H, 4, 8),
        "R": bi(R, 4, 4),
    }


def postprocess(results):
    """[per-core dicts with KG [NCHUNK, P, 8, 4, G] fp16] -> [B, 8, 4] fp32."""
    outs = []
    for r in results:
        kg = r["KG"].astype(np.float32)          # [NCHUNK, P, 8, 4, G]
        kg = kg.transpose(0, 1, 4, 2, 3).reshape(B_CORE, 8, 4)
        outs.append(kg)
    return np.concatenate(outs, axis=0)


def kernel(F, H, Sigma_previous, Q, R):
    from concourse.bass_utils import run_bass_kernel_spmd

    nc = _get_nc()
    in_maps = [
        prepare_in_map(F, H, Sigma_previous, Q, R, ci) for ci in range(NCORES)
    ]
    res = run_bass_kernel_spmd(nc, in_maps, core_ids=list(range(NCORES)))
    return postprocess(res.results)
